# revision 22
# baseline (speedup 1.0000x reference)
"""AdaLoRAWithBase Trainium2 kernel, 8-core SPMD.

Math (reference never needs the [B,D,D] per-sample layer):
  cond = LayerNorm(ada_emb) * gamma + beta
  h    = gelu(cond @ W1 + b1)
  w    = h @ W2 + b2 ;  x_a, x_b = split(w) -> [B, D, R] each
  out  = x + x @ base + sum_r t[:, r] * x_b[:, :, r]
         where t[b, r] = sum_d x[b, d] * x_a[b, d, r]

Sharding (8 cores): W2's output columns are sharded by d in r-major order.
Core k owns d-slots [128k, 128k+128): it computes x_a/x_b for those slots,
a partial t (reduced over its d-slots), and the output columns o in the same
range.  The [256, 8] t-partials are combined across cores with a collective;
everything else is local.  cond@W1 is computed redundantly on every core
(small), with LayerNorm folded in via the exact identity
  cond @ W1g + b1g = rstd*(ada @ W1g) - (rstd*mu)*(1^T W1g) + b1g
(W1g = gamma[:,None]*W1, b1g = b1 + beta@W1) so no on-chip transpose of the
normalized activations is needed; the host supplies ada^T and x^T directly.
"""
import sys
import types

import numpy as np

# --- axon NTFF profile hook (missing antenv.axon_hooks in this image) -------
if "antenv.axon_hooks" not in sys.modules:
    _mod = types.ModuleType("antenv.axon_hooks")
    _HOOK = [None]
    _mod.set_axon_ntff_profile_hook = lambda h: _HOOK.__setitem__(0, h)
    _mod.get_axon_ntff_profile_hook = lambda: _HOOK[0]
    sys.modules["antenv.axon_hooks"] = _mod
    try:
        import antenv

        antenv.axon_hooks = _mod
        if "/root/.axon_site" not in sys.path:
            sys.path.insert(0, "/root/.axon_site")
        from trn_agent_boot.trn_boot import _ntff_profile_via_ctypes

        _mod.set_axon_ntff_profile_hook(
            _ntff_profile_via_ctypes("/opt/axon/libaxon_pjrt.so")
        )
    except Exception:
        pass

if "/opt/trn_rl_repo" not in sys.path:
    sys.path.insert(0, "/opt/trn_rl_repo")

import ml_dtypes
import concourse.bass as bass
import concourse.mybir as mybir
import concourse.tile as tile
from concourse import bacc
from concourse.bass_utils import run_bass_kernel_spmd

B, D, A, I, R = 256, 1024, 1024, 1024, 8
N = 8          # cores
DS = D // N    # 128 d-slots per core
LN_EPS = 1e-5
F32 = mybir.dt.float32
BF16 = mybir.dt.bfloat16
ALU = mybir.AluOpType
ACTF = mybir.ActivationFunctionType
BF16_NP = ml_dtypes.bfloat16

_CACHE = {}


def build_graph(t_mode: str = "ar8", finalize: bool = True, debug: bool = False):
    nc = bacc.Bacc("TRN2", target_bir_lowering=False, debug=False, num_devices=N)

    # per-core external inputs (shapes are per-core shards)
    adaT_d = nc.dram_tensor("adaT", [A, B], BF16, kind="ExternalInput")      # ada^T
    xT_d = nc.dram_tensor("xT", [D, B], BF16, kind="ExternalInput")          # x^T
    xs_d = nc.dram_tensor("xs", [B, DS], F32, kind="ExternalInput")          # x[:, d-slots]
    w1_d = nc.dram_tensor("w1", [A, I], BF16, kind="ExternalInput")          # gamma-folded
    w2_d = nc.dram_tensor("w2", [I, 2 * R * DS], BF16, kind="ExternalInput") # [xa | xb] r-major
    ba_d = nc.dram_tensor("baseaug", [D, DS + R], BF16, kind="ExternalInput")# [base cols | b2a]
    b1_d = nc.dram_tensor("b1", [I], F32, kind="ExternalInput")              # beta-folded
    negS_d = nc.dram_tensor("negS", [I], F32, kind="ExternalInput")          # -colsum(W1g)
    b2b_d = nc.dram_tensor("b2brow", [1, R * DS], F32, kind="ExternalInput") # r-major
    out_d = nc.dram_tensor("out", [B, DS], F32, kind="ExternalOutput")
    if debug:
        dbg_stat_d = nc.dram_tensor("dbg_stat", [128, 2, B], F32, kind="ExternalOutput")
        dbg_h_d = nc.dram_tensor("dbg_h", [128, 8, B], BF16, kind="ExternalOutput")
        dbg_tpart_d = nc.dram_tensor("dbg_tpart", [128, 2, R], F32, kind="ExternalOutput")
        dbg_tsum_d = nc.dram_tensor("dbg_tsum", [128, 2, R], F32, kind="ExternalOutput")
        dbg_xb_d = nc.dram_tensor("dbg_xb", [128, 2, R * DS], F32, kind="ExternalOutput")

    if t_mode == "rdma":
        rsem = nc.alloc_semaphore("t_rsem")
        lsem = nc.alloc_semaphore("t_lsem")
        psem = nc.alloc_semaphore("t_psem")

    with tile.TileContext(nc) as tc:
        with tc.tile_pool(name="weights", bufs=1) as wpool, \
             tc.tile_pool(name="acts", bufs=1) as apool, \
             tc.tile_pool(name="small", bufs=1) as spool, \
             tc.tile_pool(name="scratch", bufs=3) as scr, \
             tc.tile_pool(name="psum_h", bufs=2, space="PSUM") as ph, \
             tc.tile_pool(name="psum_w", bufs=4, space="PSUM") as pw, \
             tc.tile_pool(name="psum_b", bufs=2, space="PSUM") as pb, \
             tc.tile_pool(name="dram", bufs=1, space="DRAM") as dram:

            # ---- loads. Priority: adaT (stats path), then W1 split across
            # both HWDGE queues, then W2 (x_a half first), then the rest.
            adaT_sb = apool.tile([128, 8, B], BF16)
            nc.sync.dma_start(adaT_sb[:], adaT_d.ap().rearrange("(kt p) b -> p kt b", p=128))
            w1_sb = wpool.tile([128, 8, I], BF16)
            w1_view = w1_d.ap().rearrange("(kt p) i -> p kt i", p=128)
            nc.sync.dma_start(w1_sb[:, 0:4], w1_view[:, 0:4])
            nc.scalar.dma_start(w1_sb[:, 4:8], w1_view[:, 4:8])
            w2_sb = wpool.tile([128, 8, 2 * R * DS], BF16)
            w2_view = w2_d.ap().rearrange("(kt p) c -> p kt c", p=128)
            nc.scalar.dma_start(w2_sb[:, :, 0 : 512], w2_view[:, :, 0 : 512])
            nc.sync.dma_start(w2_sb[:, :, 512 : 1024], w2_view[:, :, 512 : 1024])
            nc.scalar.dma_start(w2_sb[:, :, 1024 : 1536], w2_view[:, :, 1024 : 1536])
            nc.sync.dma_start(w2_sb[:, :, 1536 : 2048], w2_view[:, :, 1536 : 2048])
            xs_sb = apool.tile([128, 2, DS], F32)
            nc.gpsimd.dma_start(xs_sb[:], xs_d.ap().rearrange("(c p) o -> p c o", p=128))
            xT_sb = apool.tile([128, 8, B], BF16)
            nc.gpsimd.dma_start(xT_sb[:], xT_d.ap().rearrange("(kt p) b -> p kt b", p=128))
            ba_sb = wpool.tile([128, 8, DS + R], BF16)
            nc.gpsimd.dma_start(ba_sb[:], ba_d.ap().rearrange("(kt p) c -> p kt c", p=128))
            b1_sb = spool.tile([128, 8], F32)
            nc.gpsimd.dma_start(b1_sb[:], b1_d.ap().rearrange("(blk p) -> p blk", p=128))
            negS_sb = spool.tile([128, 8], F32)
            nc.gpsimd.dma_start(negS_sb[:], negS_d.ap().rearrange("(blk p) -> p blk", p=128))
            b2b_row = spool.tile([1, R * DS], F32)
            nc.gpsimd.dma_start(b2b_row[:], b2b_d.ap())
            b2b_sb = spool.tile([128, R * DS], F32)
            nc.gpsimd.partition_broadcast(b2b_sb[:], b2b_row[:])

            # ---- dummy tiny collective to warm up ncfw before the real
            # one; no input DMA (contents are irrelevant) so it fires at t~0.
            if t_mode == "ar8warm":
                dummy_in = dram.tile([1, 8], F32)
                dummy_out = dram.tile([1, 8], F32, addr_space="Shared")
                nc.gpsimd.collective_compute(
                    "AllReduce", ALU.add,
                    replica_groups=[list(range(N))],
                    ins=[dummy_in[:].opt()], outs=[dummy_out[:].opt()])

            if t_mode == "rdma":
                slots = spool.tile([128, 8, 2, R], F32)
                nc.vector.memset(slots[:], 0.0)

            # ---- LayerNorm stats from adaT via PE column sums ----
            sqT = apool.tile([128, 8, B], BF16)
            nc.vector.tensor_tensor(sqT[:], adaT_sb[:], adaT_sb[:], ALU.mult)
            ones = spool.tile([128, 1], BF16)
            nc.vector.memset(ones[:], 1.0)
            psum_s1 = pw.tile([1, B], F32, tag="pw")
            psum_s2 = pw.tile([1, B], F32, tag="pw")
            for kt in range(8):
                nc.tensor.matmul(psum_s1[:], lhsT=ones[:], rhs=adaT_sb[:, kt],
                                 start=(kt == 0), stop=(kt == 7))
            for kt in range(8):
                nc.tensor.matmul(psum_s2[:], lhsT=ones[:], rhs=sqT[:, kt],
                                 start=(kt == 0), stop=(kt == 7))
            mu = spool.tile([1, B], F32)
            nc.vector.tensor_scalar_mul(mu[:], psum_s1[:], 1.0 / A)
            musq = spool.tile([1, B], F32)
            nc.vector.tensor_tensor(musq[:], mu[:], mu[:], ALU.mult)
            vareps = spool.tile([1, B], F32)
            nc.vector.tensor_scalar(vareps[:], psum_s2[:], 1.0 / A, LN_EPS,
                                    ALU.mult, ALU.add)
            nc.vector.tensor_tensor(vareps[:], vareps[:], musq[:], ALU.subtract)
            stdv = spool.tile([1, B], F32)
            nc.scalar.activation(stdv[:], vareps[:], ACTF.Sqrt)
            strow = spool.tile([1, 2, B], F32)   # row 0 = rstd, row 1 = rstd*mu
            nc.vector.reciprocal(strow[:, 0], stdv[:])
            nc.vector.tensor_tensor(strow[:, 1], strow[:, 0], mu[:], ALU.mult)
            stat_bc = spool.tile([128, 2, B], F32)
            nc.gpsimd.partition_broadcast(stat_bc[:], strow[:])

            # ---- h = gelu(rstd*(ada @ W1g) - (rstd*mu)*S + b1g), feature-major
            h_sb = apool.tile([128, 8, B], BF16)
            for ib in range(8):
                psum_h = ph.tile([128, B], F32, tag="ph")
                for kt in range(8):
                    nc.tensor.matmul(
                        psum_h[:],
                        lhsT=w1_sb[:, kt, ib * 128 : (ib + 1) * 128],
                        rhs=adaT_sb[:, kt],
                        start=(kt == 0), stop=(kt == 7))
                hpre = scr.tile([128, B], F32, tag="hpre")
                nc.vector.tensor_tensor(hpre[:], psum_h[:], stat_bc[:, 0], ALU.mult)
                nc.vector.scalar_tensor_tensor(
                    hpre[:], in0=stat_bc[:, 1], scalar=negS_sb[:, ib : ib + 1],
                    in1=hpre[:], op0=ALU.mult, op1=ALU.add)
                nc.scalar.activation(
                    h_sb[:, ib], hpre[:], ACTF.Gelu, bias=b1_sb[:, ib : ib + 1])

            # ---- x_a half of w (kt-major, shared lhsT) -> t partials ----
            tpart = spool.tile([128, 2, R], F32)
            for c in range(2):
                pws = [pw.tile([128, 512], F32, tag="pw", name=f"pwa{c}{cb}")
                       for cb in range(2)]
                for cb in range(2):
                    for kt in range(8):
                        nc.tensor.matmul(
                            pws[cb][:],
                            lhsT=h_sb[:, kt, c * 128 : (c + 1) * 128],
                            rhs=w2_sb[:, kt, cb * 512 : (cb + 1) * 512],
                            start=(kt == 0), stop=(kt == 7))
                for cb in range(2):
                    for j in range(4):
                        r = cb * 4 + j
                        ttr_scratch = scr.tile([128, 128], F32, tag="ttr")
                        nc.vector.tensor_tensor(
                            ttr_scratch[:],
                            pws[cb][:, j * 128 : (j + 1) * 128],
                            xs_sb[:, c], ALU.mult)
                        nc.vector.tensor_reduce(
                            tpart[:, c, r : r + 1], ttr_scratch[:],
                            mybir.AxisListType.X, ALU.add)

            # ---- combine t partials across cores (early trigger) ----
            if t_mode == "rdma":
                # Direct peer SBUF writes, XOR-relative destinations:
                # receiver slot k holds the frame from core (my_id ^ k);
                # slot order is irrelevant because we sum all 8.
                nc.vector.tensor_copy(slots[:, 0], tpart[:])
                with tc.tile_critical():
                    for k in range(1, 8):
                        rd = [None] * 8
                        rd[k] = (0, k)
                        nc.gpsimd.remote_dma_broadcast(
                            slots[:, k], tpart[:], rsem, lsem, rdests=rd,
                        ).then_inc(psem, 1)
                    nc.gpsimd.wait_ge(psem, 7)
                    nc.gpsimd.trigger_dma(count=None)
                    nc.vector.wait_ge(rsem, 14)  # 2 per sender x 7 senders
                # critical exit drains every engine, so these are ordered
                # after the wait; Tile handles their internal syncs.
                tsum = spool.tile([128, 2, R], F32)
                nc.vector.tensor_tensor(
                    tsum[:], slots[:, 0], slots[:, 1], ALU.add)
                for k in range(2, 8):
                    nc.vector.tensor_tensor(
                        tsum[:], tsum[:], slots[:, k], ALU.add)
            if t_mode != "rdma":
                cc_in = dram.tile([B, R], F32)
                nc.gpsimd.dma_start(cc_in[:].rearrange("(c p) r -> p c r", p=128), tpart[:])
            if t_mode in ("ar8", "ar8warm"):
                cc_out = dram.tile([B, R], F32, addr_space="Shared")
                nc.gpsimd.collective_compute(
                    "AllReduce", ALU.add,
                    replica_groups=[list(range(N))],
                    ins=[cc_in[:].opt()], outs=[cc_out[:].opt()])
                tsum = spool.tile([128, 2, R], F32)
                nc.gpsimd.dma_start(tsum[:], cc_out[:].rearrange("(c p) r -> p c r", p=128))
            elif t_mode == "ag8":
                cc_out = dram.tile([N * B, R], F32, addr_space="Shared")
                nc.gpsimd.collective_compute(
                    "AllGather", ALU.bypass,
                    replica_groups=[list(range(N))],
                    ins=[cc_in[:].opt()], outs=[cc_out[:].opt()])
                gath = spool.tile([128, 8, 2, R], F32)
                nc.sync.dma_start(
                    gath[:], cc_out[:].rearrange("(k c p) r -> p k c r", p=128, c=2))
                tsum = spool.tile([128, 2, R], F32)
                nc.vector.tensor_tensor(tsum[:], gath[:, 0], gath[:, 1], ALU.add)
                for k in range(2, 8):
                    nc.vector.tensor_tensor(tsum[:], tsum[:], gath[:, k], ALU.add)
            elif t_mode != "rdma":
                raise ValueError(t_mode)

            if debug:
                nc.sync.dma_start(dbg_tpart_d.ap(), tpart[:])
                nc.sync.dma_start(dbg_tsum_d.ap(), tsum[:])
                nc.sync.dma_start(dbg_stat_d.ap(), stat_bc[:])
                nc.sync.dma_start(dbg_h_d.ap(), h_sb[:])

            # ---- x_b half of w (overlaps the collective) ----
            xb_sb = apool.tile([128, 2, R * DS], F32)
            for c in range(2):
                pws = [pw.tile([128, 512], F32, tag="pw", name=f"pwb{c}{cb}")
                       for cb in range(2)]
                for cb in range(2):
                    for kt in range(8):
                        nc.tensor.matmul(
                            pws[cb][:],
                            lhsT=h_sb[:, kt, c * 128 : (c + 1) * 128],
                            rhs=w2_sb[:, kt, (2 + cb) * 512 : (3 + cb) * 512],
                            start=(kt == 0), stop=(kt == 7))
                for cb in range(2):
                    off = cb * 512
                    nc.vector.tensor_tensor(
                        xb_sb[:, c, off : off + 512], pws[cb][:],
                        b2b_sb[:, off : off + 512], ALU.add)

            if debug:
                nc.sync.dma_start(dbg_xb_d.ap(), xb_sb[:])

            # ---- base matmul (+ t_bias columns) ----
            psum_base = []
            for c in range(2):
                pbase = pb.tile([128, DS + R], F32, tag="pb")
                for kt in range(8):
                    nc.tensor.matmul(
                        pbase[:],
                        lhsT=xT_sb[:, kt, c * 128 : (c + 1) * 128],
                        rhs=ba_sb[:, kt],
                        start=(kt == 0), stop=(kt == 7))
                psum_base.append(pbase)

            # ---- epilogue: out = base + resid + sum_r t_r * x_b_r ----
            for c in range(2):
                tf = spool.tile([128, R], F32, tag=f"tf{c}")
                nc.vector.tensor_tensor(
                    tf[:], tsum[:, c], psum_base[c][:, DS : DS + R], ALU.add)
                acc = scr.tile([128, DS], F32, tag="acc")
                nc.vector.tensor_tensor(
                    acc[:], psum_base[c][:, 0:DS], xs_sb[:, c], ALU.add)
                for r in range(R):
                    nc.vector.scalar_tensor_tensor(
                        acc[:], in0=xb_sb[:, c, r * DS : (r + 1) * DS],
                        scalar=tf[:, r : r + 1], in1=acc[:],
                        op0=ALU.mult, op1=ALU.add)
                nc.sync.dma_start(
                    out_d.ap().rearrange("(c p) o -> p c o", p=128)[:, c], acc[:])

    if finalize:
        nc.finalize()
    return nc


def _prep_inputs(x, ada_emb, base_layer, ln_gamma, ln_beta, W1, b1, W2, b2):
    """Host-side: fold LN affine into W1/b1, shard W2/base by d in r-major."""
    f32 = np.float32
    x = np.asarray(x, f32)
    ada = np.asarray(ada_emb, f32)
    base = np.asarray(base_layer, f32)
    W1g = (np.asarray(ln_gamma, f32)[:, None] * np.asarray(W1, f32))
    b1g = np.asarray(b1, f32) + np.asarray(ln_beta, f32) @ np.asarray(W1, f32)
    negS = -W1g.sum(axis=0)
    W2 = np.asarray(W2, f32)
    b2 = np.asarray(b2, f32)
    W2a = W2[:, : D * R].reshape(I, D, R)
    W2b = W2[:, D * R :].reshape(I, D, R)
    b2a = b2[: D * R].reshape(D, R)
    b2b = b2[D * R :].reshape(D, R)

    adaT = np.ascontiguousarray(ada.T).astype(BF16_NP)
    xT = np.ascontiguousarray(x.T).astype(BF16_NP)
    w1_bf = W1g.astype(BF16_NP)

    in_maps = []
    for k in range(N):
        sl = slice(k * DS, (k + 1) * DS)
        # r-major columns: col = r*DS + d_local
        w2a_k = np.ascontiguousarray(W2a[:, sl, :].transpose(0, 2, 1)).reshape(I, R * DS)
        w2b_k = np.ascontiguousarray(W2b[:, sl, :].transpose(0, 2, 1)).reshape(I, R * DS)
        w2_k = np.concatenate([w2a_k, w2b_k], axis=1).astype(BF16_NP)
        baseaug_k = np.concatenate([base[:, sl], b2a], axis=1).astype(BF16_NP)
        b2b_k = np.ascontiguousarray(b2b[sl, :].T).reshape(1, R * DS).astype(f32)
        in_maps.append({
            "adaT": adaT,
            "xT": xT,
            "xs": np.ascontiguousarray(x[:, sl]),
            "w1": w1_bf,
            "w2": w2_k,
            "baseaug": baseaug_k,
            "b1": b1g,
            "negS": negS,
            "b2brow": b2b_k,
        })
    return in_maps


def kernel(x, ada_emb, base_layer, ln_gamma, ln_beta, W1, b1, W2, b2,
           t_mode="ar8warm", trace=False):
    if t_mode not in _CACHE:
        _CACHE[t_mode] = build_graph(t_mode)
    nc = _CACHE[t_mode]
    in_maps = _prep_inputs(x, ada_emb, base_layer, ln_gamma, ln_beta, W1, b1, W2, b2)
    res = run_bass_kernel_spmd(nc, in_maps, core_ids=list(range(N)), trace=trace)
    out = np.concatenate([res.results[k]["out"] for k in range(N)], axis=1)
    kernel.last_exec_time_ns = res.exec_time_ns
    return out


kernel.last_exec_time_ns = None


# revision 23
# speedup vs baseline: 1.0259x; 1.0259x over previous
"""AdaLoRAWithBase Trainium2 kernel, 8-core SPMD.

Math (reference never needs the [B,D,D] per-sample layer):
  cond = LayerNorm(ada_emb) * gamma + beta
  h    = gelu(cond @ W1 + b1)
  w    = h @ W2 + b2 ;  x_a, x_b = split(w) -> [B, D, R] each
  out  = x + x @ base + sum_r t[:, r] * x_b[:, :, r]
         where t[b, r] = sum_d x[b, d] * x_a[b, d, r]

Sharding (8 cores): W2's output columns are sharded by d in r-major order.
Core k owns d-slots [128k, 128k+128): it computes x_a/x_b for those slots,
a partial t (reduced over its d-slots), and the output columns o in the same
range.  The [256, 8] t-partials are combined across cores with a collective;
everything else is local.  cond@W1 is computed redundantly on every core
(small), with LayerNorm folded in via the exact identity
  cond @ W1g + b1g = rstd*(ada @ W1g) - (rstd*mu)*(1^T W1g) + b1g
(W1g = gamma[:,None]*W1, b1g = b1 + beta@W1) so no on-chip transpose of the
normalized activations is needed; the host supplies ada^T and x^T directly.
"""
import sys
import types

import numpy as np

# --- axon NTFF profile hook (missing antenv.axon_hooks in this image) -------
if "antenv.axon_hooks" not in sys.modules:
    _mod = types.ModuleType("antenv.axon_hooks")
    _HOOK = [None]
    _mod.set_axon_ntff_profile_hook = lambda h: _HOOK.__setitem__(0, h)
    _mod.get_axon_ntff_profile_hook = lambda: _HOOK[0]
    sys.modules["antenv.axon_hooks"] = _mod
    try:
        import antenv

        antenv.axon_hooks = _mod
        if "/root/.axon_site" not in sys.path:
            sys.path.insert(0, "/root/.axon_site")
        from trn_agent_boot.trn_boot import _ntff_profile_via_ctypes

        _mod.set_axon_ntff_profile_hook(
            _ntff_profile_via_ctypes("/opt/axon/libaxon_pjrt.so")
        )
    except Exception:
        pass

if "/opt/trn_rl_repo" not in sys.path:
    sys.path.insert(0, "/opt/trn_rl_repo")

import ml_dtypes
import concourse.bass as bass
import concourse.mybir as mybir
import concourse.tile as tile
from concourse import bacc
from concourse.bass_utils import run_bass_kernel_spmd

B, D, A, I, R = 256, 1024, 1024, 1024, 8
N = 8          # cores
DS = D // N    # 128 d-slots per core
LN_EPS = 1e-5
F32 = mybir.dt.float32
BF16 = mybir.dt.bfloat16
ALU = mybir.AluOpType
ACTF = mybir.ActivationFunctionType
BF16_NP = ml_dtypes.bfloat16

_CACHE = {}


def build_graph(t_mode: str = "ar8", finalize: bool = True, debug: bool = False):
    nc = bacc.Bacc("TRN2", target_bir_lowering=False, debug=False, num_devices=N)

    # per-core external inputs (shapes are per-core shards)
    adaT_d = nc.dram_tensor("adaT", [A, B], BF16, kind="ExternalInput")      # ada^T
    xT_d = nc.dram_tensor("xT", [D, B], BF16, kind="ExternalInput")          # x^T
    xs_d = nc.dram_tensor("xs", [B, DS], F32, kind="ExternalInput")          # x[:, d-slots]
    w1_d = nc.dram_tensor("w1", [A, I], BF16, kind="ExternalInput")          # gamma-folded
    w2_d = nc.dram_tensor("w2", [I, 2 * R * DS], BF16, kind="ExternalInput") # [xa | xb] r-major
    ba_d = nc.dram_tensor("baseaug", [D, DS + R], BF16, kind="ExternalInput")# [base cols | b2a]
    b1_d = nc.dram_tensor("b1", [I], F32, kind="ExternalInput")              # beta-folded
    negS_d = nc.dram_tensor("negS", [I], F32, kind="ExternalInput")          # -colsum(W1g)
    b2b_d = nc.dram_tensor("b2brow", [1, R * DS], F32, kind="ExternalInput") # r-major
    out_d = nc.dram_tensor("out", [B, DS], F32, kind="ExternalOutput")
    if debug:
        dbg_stat_d = nc.dram_tensor("dbg_stat", [128, 2, B], F32, kind="ExternalOutput")
        dbg_h_d = nc.dram_tensor("dbg_h", [128, 8, B], BF16, kind="ExternalOutput")
        dbg_tpart_d = nc.dram_tensor("dbg_tpart", [128, 2, R], F32, kind="ExternalOutput")
        dbg_tsum_d = nc.dram_tensor("dbg_tsum", [128, 2, R], F32, kind="ExternalOutput")
        dbg_xb_d = nc.dram_tensor("dbg_xb", [128, 2, R * DS], F32, kind="ExternalOutput")

    if t_mode == "rdma":
        rsem = nc.alloc_semaphore("t_rsem")
        lsem = nc.alloc_semaphore("t_lsem")
        psem = nc.alloc_semaphore("t_psem")

    with tile.TileContext(nc) as tc:
        with tc.tile_pool(name="weights", bufs=1) as wpool, \
             tc.tile_pool(name="acts", bufs=1) as apool, \
             tc.tile_pool(name="small", bufs=1) as spool, \
             tc.tile_pool(name="scratch", bufs=3) as scr, \
             tc.tile_pool(name="psum_h", bufs=2, space="PSUM") as ph, \
             tc.tile_pool(name="psum_w", bufs=4, space="PSUM") as pw, \
             tc.tile_pool(name="psum_b", bufs=2, space="PSUM") as pb, \
             tc.tile_pool(name="dram", bufs=1, space="DRAM") as dram:

            # ---- loads. Priority: adaT (stats path), then W1 split across
            # both HWDGE queues, then W2 (x_a half first), then the rest.
            adaT_sb = apool.tile([128, 8, B], BF16)
            nc.sync.dma_start(adaT_sb[:], adaT_d.ap().rearrange("(kt p) b -> p kt b", p=128))
            w1_sb = wpool.tile([128, 8, I], BF16)
            w1_view = w1_d.ap().rearrange("(kt p) i -> p kt i", p=128)
            nc.sync.dma_start(w1_sb[:, 0:4], w1_view[:, 0:4])
            nc.scalar.dma_start(w1_sb[:, 4:8], w1_view[:, 4:8])
            w2_sb = wpool.tile([128, 8, 2 * R * DS], BF16)
            w2_view = w2_d.ap().rearrange("(kt p) c -> p kt c", p=128)
            nc.scalar.dma_start(w2_sb[:, :, 0 : 512], w2_view[:, :, 0 : 512])
            nc.sync.dma_start(w2_sb[:, :, 512 : 1024], w2_view[:, :, 512 : 1024])
            nc.scalar.dma_start(w2_sb[:, :, 1024 : 1536], w2_view[:, :, 1024 : 1536])
            nc.sync.dma_start(w2_sb[:, :, 1536 : 2048], w2_view[:, :, 1536 : 2048])
            xs_sb = apool.tile([128, 2, DS], F32)
            nc.gpsimd.dma_start(xs_sb[:], xs_d.ap().rearrange("(c p) o -> p c o", p=128))
            xT_sb = apool.tile([128, 8, B], BF16)
            nc.gpsimd.dma_start(xT_sb[:], xT_d.ap().rearrange("(kt p) b -> p kt b", p=128))
            ba_sb = wpool.tile([128, 8, DS + R], BF16)
            nc.gpsimd.dma_start(ba_sb[:], ba_d.ap().rearrange("(kt p) c -> p kt c", p=128))
            b1_sb = spool.tile([128, 8], F32)
            nc.gpsimd.dma_start(b1_sb[:], b1_d.ap().rearrange("(blk p) -> p blk", p=128))
            negS_sb = spool.tile([128, 8], F32)
            nc.gpsimd.dma_start(negS_sb[:], negS_d.ap().rearrange("(blk p) -> p blk", p=128))
            b2b_row = spool.tile([1, R * DS], F32)
            nc.gpsimd.dma_start(b2b_row[:], b2b_d.ap())
            b2b_sb = spool.tile([128, R * DS], F32)
            nc.gpsimd.partition_broadcast(b2b_sb[:], b2b_row[:])

            # ---- dummy tiny collective to warm up ncfw before the real one
            if t_mode == "ar8warm":
                dummy_in = dram.tile([1, 8], F32)
                nc.gpsimd.dma_start(dummy_in[:], b2b_d.ap()[:, 0:8])
                dummy_out = dram.tile([1, 8], F32, addr_space="Shared")
                nc.gpsimd.collective_compute(
                    "AllReduce", ALU.add,
                    replica_groups=[list(range(N))],
                    ins=[dummy_in[:].opt()], outs=[dummy_out[:].opt()])

            if t_mode == "rdma":
                slots = spool.tile([128, 8, 2, R], F32)
                nc.vector.memset(slots[:], 0.0)

            # ---- LayerNorm stats from adaT via PE column sums ----
            sqT = apool.tile([128, 8, B], BF16)
            nc.vector.tensor_tensor(sqT[:], adaT_sb[:], adaT_sb[:], ALU.mult)
            ones = spool.tile([128, 1], BF16)
            nc.vector.memset(ones[:], 1.0)
            psum_s1 = pw.tile([1, B], F32, tag="pw")
            psum_s2 = pw.tile([1, B], F32, tag="pw")
            for kt in range(8):
                nc.tensor.matmul(psum_s1[:], lhsT=ones[:], rhs=adaT_sb[:, kt],
                                 start=(kt == 0), stop=(kt == 7))
            for kt in range(8):
                nc.tensor.matmul(psum_s2[:], lhsT=ones[:], rhs=sqT[:, kt],
                                 start=(kt == 0), stop=(kt == 7))
            mu = spool.tile([1, B], F32)
            nc.vector.tensor_scalar_mul(mu[:], psum_s1[:], 1.0 / A)
            musq = spool.tile([1, B], F32)
            nc.vector.tensor_tensor(musq[:], mu[:], mu[:], ALU.mult)
            vareps = spool.tile([1, B], F32)
            nc.vector.tensor_scalar(vareps[:], psum_s2[:], 1.0 / A, LN_EPS,
                                    ALU.mult, ALU.add)
            nc.vector.tensor_tensor(vareps[:], vareps[:], musq[:], ALU.subtract)
            stdv = spool.tile([1, B], F32)
            nc.scalar.activation(stdv[:], vareps[:], ACTF.Sqrt)
            strow = spool.tile([1, 2, B], F32)   # row 0 = rstd, row 1 = rstd*mu
            nc.vector.reciprocal(strow[:, 0], stdv[:])
            nc.vector.tensor_tensor(strow[:, 1], strow[:, 0], mu[:], ALU.mult)
            stat_bc = spool.tile([128, 2, B], F32)
            nc.gpsimd.partition_broadcast(stat_bc[:], strow[:])

            # ---- h = gelu(rstd*(ada @ W1g) - (rstd*mu)*S + b1g), feature-major
            h_sb = apool.tile([128, 8, B], BF16)
            for ib in range(8):
                psum_h = ph.tile([128, B], F32, tag="ph")
                for kt in range(8):
                    nc.tensor.matmul(
                        psum_h[:],
                        lhsT=w1_sb[:, kt, ib * 128 : (ib + 1) * 128],
                        rhs=adaT_sb[:, kt],
                        start=(kt == 0), stop=(kt == 7))
                hpre = scr.tile([128, B], F32, tag="hpre")
                nc.vector.tensor_tensor(hpre[:], psum_h[:], stat_bc[:, 0], ALU.mult)
                nc.vector.scalar_tensor_tensor(
                    hpre[:], in0=stat_bc[:, 1], scalar=negS_sb[:, ib : ib + 1],
                    in1=hpre[:], op0=ALU.mult, op1=ALU.add)
                nc.scalar.activation(
                    h_sb[:, ib], hpre[:], ACTF.Gelu, bias=b1_sb[:, ib : ib + 1])

            # ---- x_a half of w (kt-major, shared lhsT) -> t partials ----
            tpart = spool.tile([128, 2, R], F32)
            for c in range(2):
                pws = [pw.tile([128, 512], F32, tag="pw", name=f"pwa{c}{cb}")
                       for cb in range(2)]
                for cb in range(2):
                    for kt in range(8):
                        nc.tensor.matmul(
                            pws[cb][:],
                            lhsT=h_sb[:, kt, c * 128 : (c + 1) * 128],
                            rhs=w2_sb[:, kt, cb * 512 : (cb + 1) * 512],
                            start=(kt == 0), stop=(kt == 7))
                for cb in range(2):
                    for j in range(4):
                        r = cb * 4 + j
                        ttr_scratch = scr.tile([128, 128], F32, tag="ttr")
                        nc.vector.tensor_tensor(
                            ttr_scratch[:],
                            pws[cb][:, j * 128 : (j + 1) * 128],
                            xs_sb[:, c], ALU.mult)
                        nc.vector.tensor_reduce(
                            tpart[:, c, r : r + 1], ttr_scratch[:],
                            mybir.AxisListType.X, ALU.add)

            # ---- combine t partials across cores (early trigger) ----
            if t_mode == "rdma":
                # Direct peer SBUF writes, XOR-relative destinations:
                # receiver slot k holds the frame from core (my_id ^ k);
                # slot order is irrelevant because we sum all 8.
                nc.vector.tensor_copy(slots[:, 0], tpart[:])
                with tc.tile_critical():
                    for k in range(1, 8):
                        rd = [None] * 8
                        rd[k] = (0, k)
                        nc.gpsimd.remote_dma_broadcast(
                            slots[:, k], tpart[:], rsem, lsem, rdests=rd,
                        ).then_inc(psem, 1)
                    nc.gpsimd.wait_ge(psem, 7)
                    nc.gpsimd.trigger_dma(count=None)
                    nc.vector.wait_ge(rsem, 14)  # 2 per sender x 7 senders
                # critical exit drains every engine, so these are ordered
                # after the wait; Tile handles their internal syncs.
                tsum = spool.tile([128, 2, R], F32)
                nc.vector.tensor_tensor(
                    tsum[:], slots[:, 0], slots[:, 1], ALU.add)
                for k in range(2, 8):
                    nc.vector.tensor_tensor(
                        tsum[:], tsum[:], slots[:, k], ALU.add)
            if t_mode != "rdma":
                cc_in = dram.tile([B, R], F32)
                nc.gpsimd.dma_start(cc_in[:].rearrange("(c p) r -> p c r", p=128), tpart[:])
            if t_mode in ("ar8", "ar8warm"):
                cc_out = dram.tile([B, R], F32, addr_space="Shared")
                nc.gpsimd.collective_compute(
                    "AllReduce", ALU.add,
                    replica_groups=[list(range(N))],
                    ins=[cc_in[:].opt()], outs=[cc_out[:].opt()])
                tsum = spool.tile([128, 2, R], F32)
                nc.gpsimd.dma_start(tsum[:], cc_out[:].rearrange("(c p) r -> p c r", p=128))
            elif t_mode == "ag8":
                cc_out = dram.tile([N * B, R], F32, addr_space="Shared")
                nc.gpsimd.collective_compute(
                    "AllGather", ALU.bypass,
                    replica_groups=[list(range(N))],
                    ins=[cc_in[:].opt()], outs=[cc_out[:].opt()])
                gath = spool.tile([128, 8, 2, R], F32)
                nc.sync.dma_start(
                    gath[:], cc_out[:].rearrange("(k c p) r -> p k c r", p=128, c=2))
                tsum = spool.tile([128, 2, R], F32)
                nc.vector.tensor_tensor(tsum[:], gath[:, 0], gath[:, 1], ALU.add)
                for k in range(2, 8):
                    nc.vector.tensor_tensor(tsum[:], tsum[:], gath[:, k], ALU.add)
            elif t_mode != "rdma":
                raise ValueError(t_mode)

            if debug:
                nc.sync.dma_start(dbg_tpart_d.ap(), tpart[:])
                nc.sync.dma_start(dbg_tsum_d.ap(), tsum[:])
                nc.sync.dma_start(dbg_stat_d.ap(), stat_bc[:])
                nc.sync.dma_start(dbg_h_d.ap(), h_sb[:])

            # ---- x_b half of w (overlaps the collective) ----
            xb_sb = apool.tile([128, 2, R * DS], F32)
            for c in range(2):
                pws = [pw.tile([128, 512], F32, tag="pw", name=f"pwb{c}{cb}")
                       for cb in range(2)]
                for cb in range(2):
                    for kt in range(8):
                        nc.tensor.matmul(
                            pws[cb][:],
                            lhsT=h_sb[:, kt, c * 128 : (c + 1) * 128],
                            rhs=w2_sb[:, kt, (2 + cb) * 512 : (3 + cb) * 512],
                            start=(kt == 0), stop=(kt == 7))
                for cb in range(2):
                    off = cb * 512
                    nc.vector.tensor_tensor(
                        xb_sb[:, c, off : off + 512], pws[cb][:],
                        b2b_sb[:, off : off + 512], ALU.add)

            if debug:
                nc.sync.dma_start(dbg_xb_d.ap(), xb_sb[:])

            # ---- base matmul (+ t_bias columns) ----
            psum_base = []
            for c in range(2):
                pbase = pb.tile([128, DS + R], F32, tag="pb")
                for kt in range(8):
                    nc.tensor.matmul(
                        pbase[:],
                        lhsT=xT_sb[:, kt, c * 128 : (c + 1) * 128],
                        rhs=ba_sb[:, kt],
                        start=(kt == 0), stop=(kt == 7))
                psum_base.append(pbase)

            # ---- epilogue: out = base + resid + sum_r t_r * x_b_r ----
            for c in range(2):
                tf = spool.tile([128, R], F32, tag=f"tf{c}")
                nc.vector.tensor_tensor(
                    tf[:], tsum[:, c], psum_base[c][:, DS : DS + R], ALU.add)
                acc = scr.tile([128, DS], F32, tag="acc")
                nc.vector.tensor_tensor(
                    acc[:], psum_base[c][:, 0:DS], xs_sb[:, c], ALU.add)
                for r in range(R):
                    nc.vector.scalar_tensor_tensor(
                        acc[:], in0=xb_sb[:, c, r * DS : (r + 1) * DS],
                        scalar=tf[:, r : r + 1], in1=acc[:],
                        op0=ALU.mult, op1=ALU.add)
                nc.sync.dma_start(
                    out_d.ap().rearrange("(c p) o -> p c o", p=128)[:, c], acc[:])

    if finalize:
        nc.finalize()
    return nc


def _prep_inputs(x, ada_emb, base_layer, ln_gamma, ln_beta, W1, b1, W2, b2):
    """Host-side: fold LN affine into W1/b1, shard W2/base by d in r-major."""
    f32 = np.float32
    x = np.asarray(x, f32)
    ada = np.asarray(ada_emb, f32)
    base = np.asarray(base_layer, f32)
    W1g = (np.asarray(ln_gamma, f32)[:, None] * np.asarray(W1, f32))
    b1g = np.asarray(b1, f32) + np.asarray(ln_beta, f32) @ np.asarray(W1, f32)
    negS = -W1g.sum(axis=0)
    W2 = np.asarray(W2, f32)
    b2 = np.asarray(b2, f32)
    W2a = W2[:, : D * R].reshape(I, D, R)
    W2b = W2[:, D * R :].reshape(I, D, R)
    b2a = b2[: D * R].reshape(D, R)
    b2b = b2[D * R :].reshape(D, R)

    adaT = np.ascontiguousarray(ada.T).astype(BF16_NP)
    xT = np.ascontiguousarray(x.T).astype(BF16_NP)
    w1_bf = W1g.astype(BF16_NP)

    in_maps = []
    for k in range(N):
        sl = slice(k * DS, (k + 1) * DS)
        # r-major columns: col = r*DS + d_local
        w2a_k = np.ascontiguousarray(W2a[:, sl, :].transpose(0, 2, 1)).reshape(I, R * DS)
        w2b_k = np.ascontiguousarray(W2b[:, sl, :].transpose(0, 2, 1)).reshape(I, R * DS)
        w2_k = np.concatenate([w2a_k, w2b_k], axis=1).astype(BF16_NP)
        baseaug_k = np.concatenate([base[:, sl], b2a], axis=1).astype(BF16_NP)
        b2b_k = np.ascontiguousarray(b2b[sl, :].T).reshape(1, R * DS).astype(f32)
        in_maps.append({
            "adaT": adaT,
            "xT": xT,
            "xs": np.ascontiguousarray(x[:, sl]),
            "w1": w1_bf,
            "w2": w2_k,
            "baseaug": baseaug_k,
            "b1": b1g,
            "negS": negS,
            "b2brow": b2b_k,
        })
    return in_maps


def kernel(x, ada_emb, base_layer, ln_gamma, ln_beta, W1, b1, W2, b2,
           t_mode="ar8warm", trace=False):
    if t_mode not in _CACHE:
        _CACHE[t_mode] = build_graph(t_mode)
    nc = _CACHE[t_mode]
    in_maps = _prep_inputs(x, ada_emb, base_layer, ln_gamma, ln_beta, W1, b1, W2, b2)
    res = run_bass_kernel_spmd(nc, in_maps, core_ids=list(range(N)), trace=trace)
    out = np.concatenate([res.results[k]["out"] for k in range(N)], axis=1)
    kernel.last_exec_time_ns = res.exec_time_ns
    return out


kernel.last_exec_time_ns = None


# revision 24
# speedup vs baseline: 1.0549x; 1.0283x over previous
"""AdaLoRAWithBase Trainium2 kernel, 8-core SPMD.

Math (reference never needs the [B,D,D] per-sample layer):
  cond = LayerNorm(ada_emb) * gamma + beta
  h    = gelu(cond @ W1 + b1)
  w    = h @ W2 + b2 ;  x_a, x_b = split(w) -> [B, D, R] each
  out  = x + x @ base + sum_r t[:, r] * x_b[:, :, r]
         where t[b, r] = sum_d x[b, d] * x_a[b, d, r]

Sharding (8 cores): W2's output columns are sharded by d in r-major order.
Core k owns d-slots [128k, 128k+128): it computes x_a/x_b for those slots,
a partial t (reduced over its d-slots), and the output columns o in the same
range.  The [256, 8] t-partials are combined across cores with a collective;
everything else is local.  cond@W1 is computed redundantly on every core
(small), with LayerNorm folded in via the exact identity
  cond @ W1g + b1g = rstd*(ada @ W1g) - (rstd*mu)*(1^T W1g) + b1g
(W1g = gamma[:,None]*W1, b1g = b1 + beta@W1) so no on-chip transpose of the
normalized activations is needed; the host supplies ada^T and x^T directly.
"""
import sys
import types

import numpy as np

# --- axon NTFF profile hook (missing antenv.axon_hooks in this image) -------
if "antenv.axon_hooks" not in sys.modules:
    _mod = types.ModuleType("antenv.axon_hooks")
    _HOOK = [None]
    _mod.set_axon_ntff_profile_hook = lambda h: _HOOK.__setitem__(0, h)
    _mod.get_axon_ntff_profile_hook = lambda: _HOOK[0]
    sys.modules["antenv.axon_hooks"] = _mod
    try:
        import antenv

        antenv.axon_hooks = _mod
        if "/root/.axon_site" not in sys.path:
            sys.path.insert(0, "/root/.axon_site")
        from trn_agent_boot.trn_boot import _ntff_profile_via_ctypes

        _mod.set_axon_ntff_profile_hook(
            _ntff_profile_via_ctypes("/opt/axon/libaxon_pjrt.so")
        )
    except Exception:
        pass

if "/opt/trn_rl_repo" not in sys.path:
    sys.path.insert(0, "/opt/trn_rl_repo")

import ml_dtypes
import concourse.bass as bass
import concourse.mybir as mybir
import concourse.tile as tile
from concourse import bacc
from concourse.bass_utils import run_bass_kernel_spmd

B, D, A, I, R = 256, 1024, 1024, 1024, 8
N = 8          # cores
DS = D // N    # 128 d-slots per core
LN_EPS = 1e-5
F32 = mybir.dt.float32
BF16 = mybir.dt.bfloat16
ALU = mybir.AluOpType
ACTF = mybir.ActivationFunctionType
BF16_NP = ml_dtypes.bfloat16

_CACHE = {}


def build_graph(t_mode: str = "ar8", finalize: bool = True, debug: bool = False):
    nc = bacc.Bacc("TRN2", target_bir_lowering=False, debug=False, num_devices=N)

    # per-core external inputs (shapes are per-core shards)
    adaT_d = nc.dram_tensor("adaT", [A, B], BF16, kind="ExternalInput")      # ada^T
    xT_d = nc.dram_tensor("xT", [D, B], BF16, kind="ExternalInput")          # x^T
    xs_d = nc.dram_tensor("xs", [B, DS], F32, kind="ExternalInput")          # x[:, d-slots]
    w1_d = nc.dram_tensor("w1", [A, I], BF16, kind="ExternalInput")          # gamma-folded
    w2_d = nc.dram_tensor("w2", [I, 2 * R * DS], BF16, kind="ExternalInput") # [xa | xb] r-major
    ba_d = nc.dram_tensor("baseaug", [D, DS + R], BF16, kind="ExternalInput")# [base cols | b2a]
    b1_d = nc.dram_tensor("b1", [I], F32, kind="ExternalInput")              # beta-folded
    negS_d = nc.dram_tensor("negS", [I], F32, kind="ExternalInput")          # -colsum(W1g)
    b2b_d = nc.dram_tensor("b2brow", [1, R * DS], F32, kind="ExternalInput") # r-major
    out_d = nc.dram_tensor("out", [B, DS], F32, kind="ExternalOutput")
    if debug:
        dbg_stat_d = nc.dram_tensor("dbg_stat", [128, 2, B], F32, kind="ExternalOutput")
        dbg_h_d = nc.dram_tensor("dbg_h", [128, 8, B], BF16, kind="ExternalOutput")
        dbg_tpart_d = nc.dram_tensor("dbg_tpart", [128, 2, R], F32, kind="ExternalOutput")
        dbg_tsum_d = nc.dram_tensor("dbg_tsum", [128, 2, R], F32, kind="ExternalOutput")
        dbg_xb_d = nc.dram_tensor("dbg_xb", [128, 2, R * DS], F32, kind="ExternalOutput")

    if t_mode == "rdma":
        rsem = nc.alloc_semaphore("t_rsem")
        lsem = nc.alloc_semaphore("t_lsem")
        psem = nc.alloc_semaphore("t_psem")

    with tile.TileContext(nc) as tc:
        with tc.tile_pool(name="weights", bufs=1) as wpool, \
             tc.tile_pool(name="acts", bufs=1) as apool, \
             tc.tile_pool(name="small", bufs=1) as spool, \
             tc.tile_pool(name="scratch", bufs=3) as scr, \
             tc.tile_pool(name="psum_h", bufs=3, space="PSUM") as ph, \
             tc.tile_pool(name="psum_w", bufs=3, space="PSUM") as pw, \
             tc.tile_pool(name="psum_b", bufs=2, space="PSUM") as pb, \
             tc.tile_pool(name="dram", bufs=1, space="DRAM") as dram:

            # ---- loads. Priority: adaT (stats path), then W1 split across
            # both HWDGE queues, then W2 (x_a half first), then the rest.
            adaT_sb = apool.tile([128, 8, B], BF16)
            nc.sync.dma_start(adaT_sb[:], adaT_d.ap().rearrange("(kt p) b -> p kt b", p=128))
            w1_sb = wpool.tile([128, 8, I], BF16)
            w1_view = w1_d.ap().rearrange("(kt p) i -> p kt i", p=128)
            nc.sync.dma_start(w1_sb[:, 0:4], w1_view[:, 0:4])
            nc.scalar.dma_start(w1_sb[:, 4:8], w1_view[:, 4:8])
            w2_sb = wpool.tile([128, 8, 2 * R * DS], BF16)
            w2_view = w2_d.ap().rearrange("(kt p) c -> p kt c", p=128)
            nc.scalar.dma_start(w2_sb[:, :, 0 : 512], w2_view[:, :, 0 : 512])
            nc.sync.dma_start(w2_sb[:, :, 512 : 1024], w2_view[:, :, 512 : 1024])
            nc.scalar.dma_start(w2_sb[:, :, 1024 : 1536], w2_view[:, :, 1024 : 1536])
            nc.sync.dma_start(w2_sb[:, :, 1536 : 2048], w2_view[:, :, 1536 : 2048])
            xs_sb = apool.tile([128, 2, DS], F32)
            nc.gpsimd.dma_start(xs_sb[:], xs_d.ap().rearrange("(c p) o -> p c o", p=128))
            xT_sb = apool.tile([128, 8, B], BF16)
            nc.gpsimd.dma_start(xT_sb[:], xT_d.ap().rearrange("(kt p) b -> p kt b", p=128))
            ba_sb = wpool.tile([128, 8, DS + R], BF16)
            nc.gpsimd.dma_start(ba_sb[:], ba_d.ap().rearrange("(kt p) c -> p kt c", p=128))
            b1_sb = spool.tile([128, 8], F32)
            nc.gpsimd.dma_start(b1_sb[:], b1_d.ap().rearrange("(blk p) -> p blk", p=128))
            negS_sb = spool.tile([128, 8], F32)
            nc.gpsimd.dma_start(negS_sb[:], negS_d.ap().rearrange("(blk p) -> p blk", p=128))
            b2b_row = spool.tile([1, R * DS], F32)
            nc.gpsimd.dma_start(b2b_row[:], b2b_d.ap())
            b2b_sb = spool.tile([128, R * DS], F32)
            nc.gpsimd.partition_broadcast(b2b_sb[:], b2b_row[:])

            # ---- dummy tiny collective to warm up ncfw before the real one
            if t_mode == "ar8warm":
                dummy_in = dram.tile([1, 8], F32)
                nc.gpsimd.dma_start(dummy_in[:], b2b_d.ap()[:, 0:8])
                dummy_out = dram.tile([1, 8], F32, addr_space="Shared")
                nc.gpsimd.collective_compute(
                    "AllReduce", ALU.add,
                    replica_groups=[list(range(N))],
                    ins=[dummy_in[:].opt()], outs=[dummy_out[:].opt()])

            if t_mode == "rdma":
                slots = spool.tile([128, 8, 2, R], F32)
                nc.vector.memset(slots[:], 0.0)

            # ---- LayerNorm stats from adaT via PE column sums ----
            sqT = apool.tile([128, 8, B], BF16)
            nc.vector.tensor_tensor(sqT[:], adaT_sb[:], adaT_sb[:], ALU.mult)
            ones = spool.tile([128, 1], BF16)
            nc.vector.memset(ones[:], 1.0)
            psum_s1 = pw.tile([1, B], F32, tag="pw")
            psum_s2 = pw.tile([1, B], F32, tag="pw")
            for kt in range(8):
                nc.tensor.matmul(psum_s1[:], lhsT=ones[:], rhs=adaT_sb[:, kt],
                                 start=(kt == 0), stop=(kt == 7))
            for kt in range(8):
                nc.tensor.matmul(psum_s2[:], lhsT=ones[:], rhs=sqT[:, kt],
                                 start=(kt == 0), stop=(kt == 7))
            mu = spool.tile([1, B], F32)
            nc.vector.tensor_scalar_mul(mu[:], psum_s1[:], 1.0 / A)
            musq = spool.tile([1, B], F32)
            nc.vector.tensor_tensor(musq[:], mu[:], mu[:], ALU.mult)
            vareps = spool.tile([1, B], F32)
            nc.vector.tensor_scalar(vareps[:], psum_s2[:], 1.0 / A, LN_EPS,
                                    ALU.mult, ALU.add)
            nc.vector.tensor_tensor(vareps[:], vareps[:], musq[:], ALU.subtract)
            stdv = spool.tile([1, B], F32)
            nc.scalar.activation(stdv[:], vareps[:], ACTF.Sqrt)
            strow = spool.tile([1, 2, B], F32)   # row 0 = rstd, row 1 = rstd*mu
            nc.vector.reciprocal(strow[:, 0], stdv[:])
            nc.vector.tensor_tensor(strow[:, 1], strow[:, 0], mu[:], ALU.mult)
            stat_bc = spool.tile([128, 2, B], F32)
            nc.gpsimd.partition_broadcast(stat_bc[:], strow[:])

            # ---- h = gelu(rstd*(ada @ W1g) - (rstd*mu)*S + b1g), feature-major
            h_sb = apool.tile([128, 8, B], BF16)
            for ib in range(8):
                psum_h = ph.tile([128, B], F32, tag="ph")
                for kt in range(8):
                    nc.tensor.matmul(
                        psum_h[:],
                        lhsT=w1_sb[:, kt, ib * 128 : (ib + 1) * 128],
                        rhs=adaT_sb[:, kt],
                        start=(kt == 0), stop=(kt == 7))
                hpre = scr.tile([128, B], F32, tag="hpre")
                nc.vector.tensor_tensor(hpre[:], psum_h[:], stat_bc[:, 0], ALU.mult)
                nc.vector.scalar_tensor_tensor(
                    hpre[:], in0=stat_bc[:, 1], scalar=negS_sb[:, ib : ib + 1],
                    in1=hpre[:], op0=ALU.mult, op1=ALU.add)
                nc.scalar.activation(
                    h_sb[:, ib], hpre[:], ACTF.Gelu, bias=b1_sb[:, ib : ib + 1])

            # ---- x_a half of w (kt-major, shared lhsT) -> t partials ----
            tpart = spool.tile([128, 2, R], F32)
            for c in range(2):
                pws = [pw.tile([128, 512], F32, tag="pw", name=f"pwa{c}{cb}")
                       for cb in range(2)]
                for cb in range(2):
                    for kt in range(8):
                        nc.tensor.matmul(
                            pws[cb][:],
                            lhsT=h_sb[:, kt, c * 128 : (c + 1) * 128],
                            rhs=w2_sb[:, kt, cb * 512 : (cb + 1) * 512],
                            start=(kt == 0), stop=(kt == 7))
                for cb in range(2):
                    for j in range(4):
                        r = cb * 4 + j
                        ttr_scratch = scr.tile([128, 128], F32, tag="ttr")
                        nc.vector.tensor_tensor(
                            ttr_scratch[:],
                            pws[cb][:, j * 128 : (j + 1) * 128],
                            xs_sb[:, c], ALU.mult)
                        nc.vector.tensor_reduce(
                            tpart[:, c, r : r + 1], ttr_scratch[:],
                            mybir.AxisListType.X, ALU.add)

            # ---- combine t partials across cores (early trigger) ----
            if t_mode == "rdma":
                # Direct peer SBUF writes, XOR-relative destinations:
                # receiver slot k holds the frame from core (my_id ^ k);
                # slot order is irrelevant because we sum all 8.
                nc.vector.tensor_copy(slots[:, 0], tpart[:])
                with tc.tile_critical():
                    for k in range(1, 8):
                        rd = [None] * 8
                        rd[k] = (0, k)
                        nc.gpsimd.remote_dma_broadcast(
                            slots[:, k], tpart[:], rsem, lsem, rdests=rd,
                        ).then_inc(psem, 1)
                    nc.gpsimd.wait_ge(psem, 7)
                    nc.gpsimd.trigger_dma(count=None)
                    nc.vector.wait_ge(rsem, 14)  # 2 per sender x 7 senders
                # critical exit drains every engine, so these are ordered
                # after the wait; Tile handles their internal syncs.
                tsum = spool.tile([128, 2, R], F32)
                nc.vector.tensor_tensor(
                    tsum[:], slots[:, 0], slots[:, 1], ALU.add)
                for k in range(2, 8):
                    nc.vector.tensor_tensor(
                        tsum[:], tsum[:], slots[:, k], ALU.add)
            if t_mode != "rdma":
                cc_in = dram.tile([B, R], F32)
                nc.gpsimd.dma_start(cc_in[:].rearrange("(c p) r -> p c r", p=128), tpart[:])
            if t_mode in ("ar8", "ar8warm"):
                cc_out = dram.tile([B, R], F32, addr_space="Shared")
                nc.gpsimd.collective_compute(
                    "AllReduce", ALU.add,
                    replica_groups=[list(range(N))],
                    ins=[cc_in[:].opt()], outs=[cc_out[:].opt()])
                tsum = spool.tile([128, 2, R], F32)
                nc.gpsimd.dma_start(tsum[:], cc_out[:].rearrange("(c p) r -> p c r", p=128))
            elif t_mode == "ag8":
                cc_out = dram.tile([N * B, R], F32, addr_space="Shared")
                nc.gpsimd.collective_compute(
                    "AllGather", ALU.bypass,
                    replica_groups=[list(range(N))],
                    ins=[cc_in[:].opt()], outs=[cc_out[:].opt()])
                gath = spool.tile([128, 8, 2, R], F32)
                nc.sync.dma_start(
                    gath[:], cc_out[:].rearrange("(k c p) r -> p k c r", p=128, c=2))
                tsum = spool.tile([128, 2, R], F32)
                nc.vector.tensor_tensor(tsum[:], gath[:, 0], gath[:, 1], ALU.add)
                for k in range(2, 8):
                    nc.vector.tensor_tensor(tsum[:], tsum[:], gath[:, k], ALU.add)
            elif t_mode != "rdma":
                raise ValueError(t_mode)

            if debug:
                nc.sync.dma_start(dbg_tpart_d.ap(), tpart[:])
                nc.sync.dma_start(dbg_tsum_d.ap(), tsum[:])
                nc.sync.dma_start(dbg_stat_d.ap(), stat_bc[:])
                nc.sync.dma_start(dbg_h_d.ap(), h_sb[:])

            # ---- x_b half of w (overlaps the collective) ----
            xb_sb = apool.tile([128, 2, R * DS], F32)
            for c in range(2):
                pws = [pw.tile([128, 512], F32, tag="pw", name=f"pwb{c}{cb}")
                       for cb in range(2)]
                for cb in range(2):
                    for kt in range(8):
                        nc.tensor.matmul(
                            pws[cb][:],
                            lhsT=h_sb[:, kt, c * 128 : (c + 1) * 128],
                            rhs=w2_sb[:, kt, (2 + cb) * 512 : (3 + cb) * 512],
                            start=(kt == 0), stop=(kt == 7))
                for cb in range(2):
                    off = cb * 512
                    nc.vector.tensor_tensor(
                        xb_sb[:, c, off : off + 512], pws[cb][:],
                        b2b_sb[:, off : off + 512], ALU.add)

            if debug:
                nc.sync.dma_start(dbg_xb_d.ap(), xb_sb[:])

            # ---- base matmul (+ t_bias columns) ----
            psum_base = []
            for c in range(2):
                pbase = pb.tile([128, DS + R], F32, tag="pb")
                for kt in range(8):
                    nc.tensor.matmul(
                        pbase[:],
                        lhsT=xT_sb[:, kt, c * 128 : (c + 1) * 128],
                        rhs=ba_sb[:, kt],
                        start=(kt == 0), stop=(kt == 7))
                psum_base.append(pbase)

            # ---- epilogue: out = base + resid + sum_r t_r * x_b_r ----
            for c in range(2):
                tf = spool.tile([128, R], F32, tag=f"tf{c}")
                nc.vector.tensor_tensor(
                    tf[:], tsum[:, c], psum_base[c][:, DS : DS + R], ALU.add)
                acc = scr.tile([128, DS], F32, tag="acc")
                nc.vector.tensor_tensor(
                    acc[:], psum_base[c][:, 0:DS], xs_sb[:, c], ALU.add)
                for r in range(R):
                    nc.vector.scalar_tensor_tensor(
                        acc[:], in0=xb_sb[:, c, r * DS : (r + 1) * DS],
                        scalar=tf[:, r : r + 1], in1=acc[:],
                        op0=ALU.mult, op1=ALU.add)
                nc.sync.dma_start(
                    out_d.ap().rearrange("(c p) o -> p c o", p=128)[:, c], acc[:])

    if finalize:
        nc.finalize()
    return nc


def _prep_inputs(x, ada_emb, base_layer, ln_gamma, ln_beta, W1, b1, W2, b2):
    """Host-side: fold LN affine into W1/b1, shard W2/base by d in r-major."""
    f32 = np.float32
    x = np.asarray(x, f32)
    ada = np.asarray(ada_emb, f32)
    base = np.asarray(base_layer, f32)
    W1g = (np.asarray(ln_gamma, f32)[:, None] * np.asarray(W1, f32))
    b1g = np.asarray(b1, f32) + np.asarray(ln_beta, f32) @ np.asarray(W1, f32)
    negS = -W1g.sum(axis=0)
    W2 = np.asarray(W2, f32)
    b2 = np.asarray(b2, f32)
    W2a = W2[:, : D * R].reshape(I, D, R)
    W2b = W2[:, D * R :].reshape(I, D, R)
    b2a = b2[: D * R].reshape(D, R)
    b2b = b2[D * R :].reshape(D, R)

    adaT = np.ascontiguousarray(ada.T).astype(BF16_NP)
    xT = np.ascontiguousarray(x.T).astype(BF16_NP)
    w1_bf = W1g.astype(BF16_NP)

    in_maps = []
    for k in range(N):
        sl = slice(k * DS, (k + 1) * DS)
        # r-major columns: col = r*DS + d_local
        w2a_k = np.ascontiguousarray(W2a[:, sl, :].transpose(0, 2, 1)).reshape(I, R * DS)
        w2b_k = np.ascontiguousarray(W2b[:, sl, :].transpose(0, 2, 1)).reshape(I, R * DS)
        w2_k = np.concatenate([w2a_k, w2b_k], axis=1).astype(BF16_NP)
        baseaug_k = np.concatenate([base[:, sl], b2a], axis=1).astype(BF16_NP)
        b2b_k = np.ascontiguousarray(b2b[sl, :].T).reshape(1, R * DS).astype(f32)
        in_maps.append({
            "adaT": adaT,
            "xT": xT,
            "xs": np.ascontiguousarray(x[:, sl]),
            "w1": w1_bf,
            "w2": w2_k,
            "baseaug": baseaug_k,
            "b1": b1g,
            "negS": negS,
            "b2brow": b2b_k,
        })
    return in_maps


def kernel(x, ada_emb, base_layer, ln_gamma, ln_beta, W1, b1, W2, b2,
           t_mode="ar8warm", trace=False):
    if t_mode not in _CACHE:
        _CACHE[t_mode] = build_graph(t_mode)
    nc = _CACHE[t_mode]
    in_maps = _prep_inputs(x, ada_emb, base_layer, ln_gamma, ln_beta, W1, b1, W2, b2)
    res = run_bass_kernel_spmd(nc, in_maps, core_ids=list(range(N)), trace=trace)
    out = np.concatenate([res.results[k]["out"] for k in range(N)], axis=1)
    kernel.last_exec_time_ns = res.exec_time_ns
    return out


kernel.last_exec_time_ns = None
